# revision 36
# baseline (speedup 1.0000x reference)
"""GAT message-passing layer on 8 Trainium2 NeuronCores.

Sharding: destination-node sharding (core c owns dst nodes [c*5000,(c+1)*5000)),
so all segment softmax/sums are core-local. Instead of an AllGather of k/v,
every core redundantly computes a per-core COMPACTED k/v table holding only
the src nodes its own edges reference, ordered [A-only | shared | B-only]
where A = src nodes referenced by dst-tiles 0..19 and B = by tiles 20..39.
Gathers for half A use table window [0, CAP_AO+CAP_BOTH) and for half B
[CAP_AO, rows) so every index fits the int16 limit of dma_gather.

Edge stage: CSR node-per-partition (nodes degree-sorted into 40 tiles of 128
slots). ONE batched dma_gather per tile fetches all st*128 edge kv rows
(512B each), amortizing the ~1us SWDGE fixed overhead that dominated the
per-step indirect-DMA baseline.

Feature columns are permuted head-interleaved (position dh*H + h holds
natural column h*DH + dh) end-to-end, so the per-edge dot-product reduce
runs as a stride-friendly tree over dh with h contiguous innermost --
eligible for the DVE 4x mode via scalar_tensor_tensor. Softmax uses exp
without max-subtraction (|score| < ~0.2; shift-invariant), pad slots gather
a zero row and are corrected by subtracting the pad count from the
denominator. LayerNorm's 1/sqrt(var) is exp(-0.5*ln(var+eps)) and PReLU is
the parametric_relu activation, keeping every activation in ONE table set
(natural_log_exp_and_others) -- no table reloads.
"""

import math
import sys

sys.path.insert(0, "/opt/trn_rl_repo")

import numpy as np

import concourse.bass as bass
import concourse.tile as tile
from concourse import bacc, mybir
from concourse.masks import make_identity

F32 = mybir.dt.float32
BF16 = mybir.dt.bfloat16
I16 = mybir.dt.int16
AF = mybir.ActivationFunctionType
OP = mybir.AluOpType
AX = mybir.AxisListType

N, E, D, H = 40000, 640000, 128, 8
DH = D // H
NCORES = 8
NLOC = N // NCORES          # dst nodes per core
P = 128
NT = (NLOC + P - 1) // P    # node tiles per core (40)
NTA = NT // 2               # tiles in half A
NSLOT = NT * P
SCALE = 1.0 / math.sqrt(DH * H)
EPS = 1e-5
DEN_EPS = 1e-12

# head-interleaved column permutation: permuted col i holds natural col
# (i % H) * DH + i // H  (i.e. layout [dh, h], h contiguous innermost)
PERM = (np.arange(D) % H) * DH + np.arange(D) // H


def _host_prep(src, dst):
    """Bucket edges by dst core, degree-sort nodes, build the per-core
    compacted table order + window-relative int16 gather index streams."""
    core_of = dst // NLOC
    per_core_raw = []
    for c in range(NCORES):
        m = core_of == c
        e_src = src[m]
        d_loc = dst[m] - c * NLOC
        deg = np.bincount(d_loc, minlength=NLOC)
        order = np.argsort(-deg, kind="stable")        # local ids, degree desc
        slot_of = np.empty(NLOC, np.int64)
        slot_of[order] = np.arange(NLOC)
        eslot = slot_of[d_loc]
        o2 = np.argsort(eslot, kind="stable")
        src_sorted = e_src[o2].astype(np.int64)
        eslot_sorted = eslot[o2]
        counts = np.zeros(NSLOT, np.int64)
        counts[:NLOC] = deg[order]
        per_core_raw.append((src_sorted, eslot_sorted, counts, order))

    steps = []
    for t in range(NT):
        mx = 1
        for _, _, counts, _ in per_core_raw:
            mx = max(mx, int(counts[t * P:(t + 1) * P].max()))
        steps.append(mx)
    s_total = int(np.sum(steps))
    col0 = np.zeros(NT, np.int64)
    np.cumsum(steps[:-1], out=col0[1:])

    # per-core distinct src sets per half
    infos = []
    for c in range(NCORES):
        src_sorted, eslot_sorted, counts, order = per_core_raw[c]
        etile = eslot_sorted // P
        in_a = etile < NTA
        sa = np.unique(src_sorted[in_a])
        sb = np.unique(src_sorted[~in_a])
        both = np.intersect1d(sa, sb, assume_unique=True)
        a_only = np.setdiff1d(sa, both, assume_unique=True)
        b_only = np.setdiff1d(sb, both, assume_unique=True)
        infos.append((a_only, both, b_only))

    def cap(n):
        return int((n + 64 + 127) // 128 * 128)

    cap_ao = cap(max(len(i[0]) for i in infos))
    cap_bt = cap(max(len(i[1]) for i in infos))
    cap_bo = cap(max(len(i[2]) for i in infos))
    # pad B-only so the phase-K chunk loop (8 node-tiles per chunk) divides
    while (cap_ao + cap_bt + cap_bo) % (8 * 128) != 0:
        cap_bo += 128
    tbl_rows = cap_ao + cap_bt + cap_bo
    assert cap_ao + cap_bt <= 32768 and cap_bt + cap_bo <= 32768, (
        cap_ao, cap_bt, cap_bo)
    pad_row = cap_ao + cap_bt - 1          # a guaranteed-zero row in `both` pad

    per_core = []
    for c in range(NCORES):
        src_sorted, eslot_sorted, counts, order = per_core_raw[c]
        a_only, both, b_only = infos[c]
        assert len(both) < cap_bt          # pad_row really is a pad
        row_of = np.full(N, -1, np.int64)
        row_of[a_only] = np.arange(len(a_only))
        row_of[both] = cap_ao + np.arange(len(both))
        row_of[b_only] = cap_ao + cap_bt + np.arange(len(b_only))
        tbl_nodes = np.full(tbl_rows, -1, np.int64)
        tbl_nodes[:len(a_only)] = a_only
        tbl_nodes[cap_ao:cap_ao + len(both)] = both
        tbl_nodes[cap_ao + cap_bt:cap_ao + cap_bt + len(b_only)] = b_only

        # edge stream positions
        etile = eslot_sorted // P
        ep = eslot_sorted % P
        estep = np.arange(len(src_sorted)) - np.concatenate(
            ([0], np.cumsum(np.bincount(eslot_sorted, minlength=NSLOT))))[
                eslot_sorted]
        pos = (col0[etile] + estep) * P + ep
        idx_stream = np.full(s_total * P, 0, np.int64)
        # pads: window-relative pad row (depends on tile half)
        for t in range(NT):
            lo, hi = col0[t] * P, (col0[t] + steps[t]) * P
            idx_stream[lo:hi] = pad_row if t < NTA else pad_row - cap_ao
        rows = row_of[src_sorted]
        base = np.where(etile < NTA, 0, cap_ao)
        idx_stream[pos] = rows - base
        assert idx_stream.min() >= 0 and idx_stream.max() < 32768
        eidx16 = idx_stream.reshape(s_total * P // 16, 16).T.astype(np.int16)
        eidx = np.tile(eidx16, (8, 1))     # [128, 8*s_total] replicated

        npads = np.zeros((P, NT), np.float32)
        for t in range(NT):
            npads[:, t] = steps[t] - counts[t * P:(t + 1) * P]

        slot_node = np.full(NSLOT, -1, np.int64)
        slot_node[:NLOC] = order
        sn = slot_node.reshape(NT, P).T                # [P, NT]
        perm_in = np.where(sn >= 0, c * NLOC + sn, c * NLOC).astype(np.int64)
        perm_out = np.where(sn >= 0, sn, NLOC).astype(np.int64)
        per_core.append(dict(eidx=eidx, npads=npads, tbl_nodes=tbl_nodes,
                             perm_in=perm_in, perm_out=perm_out))
    return steps, (cap_ao, cap_bt, cap_bo), per_core


def _build_program(steps, caps, ln_trivial1, ln_trivial2, b1_zero, b2_zero,
                   pw_uniform):
    s_total = int(np.sum(steps))
    cap_ao, cap_bt, cap_bo = caps
    tbl_rows = cap_ao + cap_bt + cap_bo
    win_a = cap_ao + cap_bt
    nc = bacc.Bacc("TRN2", target_bir_lowering=False, debug=False,
                   num_devices=NCORES)

    tblT_d = nc.dram_tensor("tblT", [P, tbl_rows], BF16, kind="ExternalInput").ap()
    eidx_d = nc.dram_tensor("eidx", [P, 8 * s_total], I16,
                            kind="ExternalInput").ap()
    fpm_d = nc.dram_tensor("fpm", [P, NT * D], F32, kind="ExternalInput").ap()
    fpmT_d = nc.dram_tensor("fpmT", [P, NSLOT], BF16, kind="ExternalInput").ap()
    npad_d = nc.dram_tensor("npads", [P, NT], F32, kind="ExternalInput").ap()
    wkv_d = nc.dram_tensor("wkv", [P, 2 * D], BF16, kind="ExternalInput").ap()
    wq_d = nc.dram_tensor("wq", [P, D], BF16, kind="ExternalInput").ap()
    w1_d = nc.dram_tensor("w1", [P, 4 * D], BF16, kind="ExternalInput").ap()
    w2_d = nc.dram_tensor("w2", [P, 4 * D], BF16, kind="ExternalInput").ap()
    b1_d = nc.dram_tensor("b1t", [P, 4], F32, kind="ExternalInput").ap()
    b2_d = nc.dram_tensor("b2t", [P, 1], F32, kind="ExternalInput").ap()
    pw_d = nc.dram_tensor("pwt", [P, 4], F32, kind="ExternalInput").ap()
    ln_d = {}
    for nm in ("ln1_g", "ln1_b", "ln2_g", "ln2_b"):
        ln_d[nm] = nc.dram_tensor(nm, [D], F32, kind="ExternalInput").ap()
    out_d = nc.dram_tensor("out", [P, NT * D], F32, kind="ExternalOutput").ap()

    with tile.TileContext(nc) as tc:
        consts = tc.alloc_tile_pool(name="consts", bufs=1)
        dramp = tc.alloc_tile_pool(name="dram", bufs=1, space="DRAM")
        tbl = dramp.tile([tbl_rows, 2 * D], BF16)

        ident = consts.tile([P, P], F32)
        make_identity(nc, ident[:])
        identb = consts.tile([P, P], BF16)
        nc.vector.tensor_copy(identb[:], ident[:])
        wkv = consts.tile([P, 2 * D], BF16)
        nc.sync.dma_start(wkv[:], wkv_d[:])
        wq = consts.tile([P, D], BF16)
        nc.sync.dma_start(wq[:], wq_d[:])
        w1 = consts.tile([P, 4 * D], BF16)
        nc.sync.dma_start(w1[:], w1_d[:])
        w2 = consts.tile([P, 4 * D], BF16)
        nc.sync.dma_start(w2[:], w2_d[:])
        pwt = consts.tile([P, 4], F32)
        nc.sync.dma_start(pwt[:], pw_d[:])
        b1t = consts.tile([P, 4], F32)
        nc.sync.dma_start(b1t[:], b1_d[:])
        b2t = consts.tile([P, 1], F32)
        nc.sync.dma_start(b2t[:], b2_d[:])
        npad = consts.tile([P, NT], F32)
        nc.sync.dma_start(npad[:], npad_d[:])
        eidx = consts.tile([P, 8 * s_total], I16)
        nc.sync.dma_start(eidx[:], eidx_d[:])
        epsb = consts.tile([P, 1], F32)
        nc.vector.memset(epsb[:], EPS)

        # ln gamma/beta replicated across partitions via K=1 matmul (only if
        # nontrivial), in permuted column order (host permutes).
        ln_rep = {}
        if not (ln_trivial1 and ln_trivial2):
            ones_col = consts.tile([1, P], F32)
            nc.vector.memset(ones_col[:], 1.0)
            with tc.tile_pool(name="lnpsum", bufs=1, space="PSUM") as lps:
                for nm, trivial in (("ln1_g", ln_trivial1), ("ln1_b", ln_trivial1),
                                    ("ln2_g", ln_trivial2), ("ln2_b", ln_trivial2)):
                    if trivial:
                        continue
                    row = consts.tile([1, D], F32, tag=f"row_{nm}")
                    nc.sync.dma_start(row[:], ln_d[nm][None, :])
                    ps = lps.tile([P, D], F32, tag=f"ps_{nm}")
                    nc.tensor.matmul(ps[:], lhsT=ones_col[:], rhs=row[:],
                                     start=True, stop=True)
                    rep = consts.tile([P, D], F32, tag=f"rep_{nm}")
                    nc.scalar.copy(rep[:], ps[:])
                    ln_rep[nm] = rep

        resid = tc.alloc_tile_pool(name="resid", bufs=1)
        q_all = resid.tile([P, NT * D], BF16)

        # ---------------- Phase Q: q = feat @ Wq (slot layout) ----------------
        with tc.tile_pool(name="qin", bufs=2) as qin, \
             tc.tile_pool(name="qps", bufs=2, space="PSUM") as qps:
            for tq in range(NT // 8):
                fT8 = qin.tile([P, 8 * P], BF16, tag="fT8")
                nc.sync.dma_start(fT8[:], fpmT_d[:, tq * 8 * P:(tq + 1) * 8 * P])
                for j in range(8):
                    t = tq * 8 + j
                    psq = qps.tile([P, D], F32, tag="psq")
                    nc.tensor.matmul(psq[:], lhsT=fT8[:, j * P:(j + 1) * P],
                                     rhs=wq[:], start=True, stop=True)
                    nc.scalar.copy(q_all[:, t * D:(t + 1) * D], psq[:])

        # ---------------- Phase K: compacted k/v table ----------------
        # 8 node-tiles per chunk: 1 feat-T load, 8 matmuls, 4 copies, 1 write
        KT = tbl_rows // P                  # node tiles (rows/128)
        CH = 8
        assert KT % CH == 0
        last_write_a = [None]
        last_write = [None]
        wa_chunk = (win_a // P - 1) // CH    # chunk idx containing row win_a-1
        with tc.tile_pool(name="kin", bufs=4) as kin, \
             tc.tile_pool(name="kps", bufs=4, space="PSUM") as kps, \
             tc.tile_pool(name="kout", bufs=3) as kout:
            for i in range(KT // CH):
                r0 = i * CH * P
                fch = kin.tile([P, CH * P], BF16, tag="fch")
                nc.sync.dma_start(fch[:], tblT_d[:, r0:r0 + CH * P])
                kvsb = kout.tile([P, CH * 2 * D], BF16, tag="kvsb")
                for half in range(4):
                    pkv = kps.tile([P, 2 * 2 * D], F32, tag="pkv")
                    for j in range(2):
                        jj = half * 2 + j
                        nc.tensor.matmul(
                            pkv[:, j * 2 * D:(j + 1) * 2 * D],
                            lhsT=fch[:, jj * P:(jj + 1) * P], rhs=wkv[:],
                            start=True, stop=True)
                    dsl = slice(half * 2 * 2 * D, (half + 1) * 2 * 2 * D)
                    if half < 2:
                        nc.vector.tensor_copy(kvsb[:, dsl], pkv[:])
                    else:
                        nc.scalar.copy(kvsb[:, dsl], pkv[:])
                w_inst = nc.scalar.dma_start(
                    tbl[r0: r0 + CH * P, :].rearrange("(j p) d -> p j d", p=P),
                    kvsb[:].rearrange("p (j d) -> p j d", j=CH))
                last_write[0] = w_inst
                if i == wa_chunk:
                    last_write_a[0] = last_write[0]

        # ---------------- Phase E + FFN, blocks of NTB tiles ----------------
        # LayerNorm Ln/Exp (rsqrt) batched per block so the activation table
        # only swaps 4x per block; blocks software-pipelined (E of block b+1
        # issues before FFN of block b) to keep DVE fed through LN/FFN.
        BLKS = [12, 12, 10, 6]
        assert sum(BLKS) == NT
        NTB = max(BLKS)
        B0 = np.zeros(len(BLKS) + 1, np.int64)
        np.cumsum(BLKS, out=B0[1:])
        smax = max(steps)
        col0 = np.zeros(NT, np.int64)
        np.cumsum(steps[:-1], out=col0[1:])
        with tc.tile_pool(name="egath", bufs=3) as egath, \
             tc.tile_pool(name="ework", bufs=1) as ework, \
             tc.tile_pool(name="ewvt", bufs=2) as ewvt, \
             tc.tile_pool(name="esm", bufs=2) as esm, \
             tc.tile_pool(name="eacc", bufs=2, space="PSUM") as eacc, \
             tc.tile_pool(name="eblk", bufs=2) as eblk, \
             tc.tile_pool(name="fps", bufs=1, space="PSUM") as fps, \
             tc.tile_pool(name="fh", bufs=2, space="PSUM") as fhps, \
             tc.tile_pool(name="ftmp", bufs=2) as ftmp:
            blk_state = {}
            pend = [None]

            def flush_tail():
                if pend[0] is None:
                    return
                acc, sexp, t, x2blk, vepsA, meanA, fpmblk, i = pend[0]
                pend[0] = None
                # denominator (pad-corrected)
                den = esm.tile([P, H], F32, tag="den")
                nc.vector.tensor_reduce(
                    den[:], sexp.rearrange("p (s h) -> p h s", h=H),
                    axis=AX.X, op=OP.add)
                dent = esm.tile([P, H], F32, tag="dent")
                nc.vector.tensor_scalar(out=dent[:], in0=den[:],
                                        scalar1=npad[:, t:t + 1],
                                        scalar2=DEN_EPS,
                                        op0=OP.subtract, op1=OP.add)
                rden = esm.tile([P, H], F32, tag="rden")
                nc.vector.reciprocal(rden[:], dent[:])
                xt = esm.tile([P, D], F32, tag="xt")
                nc.vector.scalar_tensor_tensor(
                    out=xt[:].rearrange("p (dh h) -> p dh h", h=H),
                    in0=acc[:].rearrange("p (dh h) -> p dh h", h=H),
                    scalar=0.0,
                    in1=rden[:].rearrange("p (o h) -> p o h", o=1)
                        .to_broadcast([P, DH, H]),
                    op0=OP.bypass, op1=OP.mult)
                isl = slice(i * D, (i + 1) * D)
                nc.vector.tensor_tensor(out=x2blk[:, isl], in0=xt[:],
                                        in1=fpmblk[:, isl], op=OP.add)
                _ln_stats(nc, esm, x2blk[:, isl], vepsA[:, i:i + 1],
                          meanA[:, i:i + 1])

            def emit_E(blk):
                nb = BLKS[blk]
                t0 = int(B0[blk])
                fpmblk = eblk.tile([P, NTB * D], F32, tag="fpmblk")
                nc.sync.dma_start(
                    fpmblk[:, :nb * D], fpm_d[:, t0 * D:(t0 + nb) * D])
                x2blk = eblk.tile([P, NTB * D], F32, tag="x2blk")
                vepsA = eblk.tile([P, NTB], F32, tag="vepsA")
                meanA = eblk.tile([P, NTB], F32, tag="meanA")
                for i in range(nb):
                    t = t0 + i
                    st = steps[t]
                    col = int(col0[t])
                    # batched gathers (<=1024 rows each: SWDGE ring limit)
                    kvb_f = egath.tile([P, smax * 2 * D], BF16, tag="kvb")
                    kvb = kvb_f[:, :st * 2 * D]
                    win = (tbl[0:win_a, :] if t < NTA
                           else tbl[cap_ao:tbl_rows, :])
                    dep = last_write_a[0] if t < NTA else last_write[0]
                    g0 = 0
                    while g0 < st:
                        ng = min(8, st - g0)
                        g_inst = nc.gpsimd.dma_gather(
                            kvb_f[:, g0 * 2 * D:(g0 + ng) * 2 * D]
                            .rearrange("p (s e) -> p s e", e=2 * D),
                            win, eidx[:, (col + g0) * 8:(col + g0 + ng) * 8],
                            ng * P, ng * P, 2 * D)
                        tile.add_dep_helper(
                            getattr(g_inst, "ins", g_inst),
                            getattr(dep, "ins", dep),
                            reason="gather after table write")
                        g0 += ng
                    kv3 = kvb.rearrange("p (s c d) -> p s c d", c=2, d=D)
                    qv = q_all[:, t * D:(t + 1) * D].rearrange(
                        "p (o d) -> p o d", o=1)
                    # prod = k * q  (TT, 2x mode)
                    prod_f = ework.tile([P, smax * D], BF16, tag="prod")
                    prod = prod_f[:, :st * D].rearrange(
                        "p (s d) -> p s d", d=D)
                    nc.vector.tensor_tensor(
                        out=prod, in0=kv3[:, :, 0],
                        in1=qv.to_broadcast([P, st, D]), op=OP.mult)
                    # tree-reduce over dh: dh-major layout keeps each level a
                    # contiguous halving with h packed innermost (TT 2x)
                    tr1_f = ework.tile([P, smax * 64], BF16, tag="tr1")
                    t1v = tr1_f[:, :st * 64].rearrange(
                        "p (s d) -> p s d", d=64)
                    nc.vector.tensor_tensor(
                        out=t1v, in0=prod[:, :, 0:64],
                        in1=prod[:, :, 64:128], op=OP.add)
                    tr2_f = ework.tile([P, smax * 32], BF16, tag="tr2")
                    t2v = tr2_f[:, :st * 32].rearrange(
                        "p (s d) -> p s d", d=32)
                    nc.vector.tensor_tensor(
                        out=t2v, in0=t1v[:, :, 0:32],
                        in1=t1v[:, :, 32:64], op=OP.add)
                    tr3_f = ework.tile([P, smax * 16], BF16, tag="tr3")
                    t3v = tr3_f[:, :st * 16].rearrange(
                        "p (s d) -> p s d", d=16)
                    nc.vector.tensor_tensor(
                        out=t3v, in0=t2v[:, :, 0:16],
                        in1=t2v[:, :, 16:32], op=OP.add)
                    scr_f = esm.tile([P, smax * H], BF16, tag="scr")
                    scr = scr_f[:, :st * H]
                    nc.vector.tensor_tensor(
                        out=scr.rearrange("p (s h) -> p s h", h=H),
                        in0=t3v[:, :, 0:8], in1=t3v[:, :, 8:16], op=OP.add)
                    # softmax numerator (no max-shift; |score| small)
                    sexp_f = esm.tile([P, smax * H], BF16, tag="sexp")
                    sexp = sexp_f[:, :st * H]
                    nc.scalar.activation(sexp, scr, AF.Exp, scale=SCALE)
                    # weighted v (TT 2x; 4D broadcast of sexp over dh)
                    sev = sexp.rearrange("p (s o h) -> p s o h", o=1, h=H)
                    wvt_f = ewvt.tile([P, smax * D], BF16, tag="wvt")
                    nc.vector.tensor_tensor(
                        out=wvt_f[:, :st * D].rearrange(
                            "p (s dh h) -> p s dh h", dh=DH, h=H),
                        in0=kv3[:, :, 1].rearrange(
                            "p s (dh h) -> p s dh h", h=H),
                        in1=sev.to_broadcast([P, st, DH, H]), op=OP.mult)
                    # accumulate sum_s wvt via PE (identity lhsT, PSUM accum)
                    acc = eacc.tile([P, D], F32, tag="acc")
                    for ss in range(st):
                        nc.tensor.matmul(acc[:], lhsT=identb[:],
                                         rhs=wvt_f[:, ss * D:(ss + 1) * D],
                                         start=(ss == 0), stop=(ss == st - 1))
                    # defer everything that waits on exp (Act) or the PE
                    # accumulation by one tile so DVE never stalls on them
                    flush_tail()
                    pend[0] = (acc, sexp, t, x2blk, vepsA, meanA, fpmblk, i)
                blk_state[blk] = (x2blk, vepsA, meanA)

            def emit_rest(blk):
                flush_tail()
                nb = BLKS[blk]
                t0 = int(B0[blk])
                x2blk, vepsA, meanA = blk_state.pop(blk)
                # batched LN1 rsqrt: r = exp(-0.5*ln(var+eps))
                rA = _ln_rsqrt(nc, eblk, vepsA[:, :nb], "A")
                bA = eblk.tile([P, NTB], F32, tag="bA")
                nc.vector.tensor_tensor(out=bA[:, :nb], in0=meanA[:, :nb],
                                        in1=rA[:], op=OP.mult)
                rstblk = eblk.tile([P, NTB * D], F32, tag="rstblk")
                for i in range(nb):
                    isl = slice(i * D, (i + 1) * D)
                    _ln_norm(nc, esm, x2blk[:, isl], rstblk[:, isl],
                             bA[:, i:i + 1], rA[:, i:i + 1],
                             ln_rep.get("ln1_g"), ln_rep.get("ln1_b"))

                # FFN (transposed space) for the block
                x2fblk = eblk.tile([P, NTB * D], F32, tag="x2fblk")
                vepsB = eblk.tile([P, NTB], F32, tag="vepsB")
                meanB = eblk.tile([P, NTB], F32, tag="meanB")
                for i in range(nb):
                    isl = slice(i * D, (i + 1) * D)
                    psr = fps.tile([P, D], F32, tag="psr")
                    nc.tensor.transpose(psr[:], rstblk[:, isl], ident[:])
                    rT = ftmp.tile([P, D], BF16, tag="rT")
                    nc.scalar.copy(rT[:], psr[:])
                    psh = fhps.tile([P, 4 * D], F32, tag="psh")
                    for cb in range(4):
                        nc.tensor.matmul(psh[:, cb * D:(cb + 1) * D],
                                         lhsT=w1[:, cb * D:(cb + 1) * D],
                                         rhs=rT[:], start=True, stop=True)
                    hsb = ftmp.tile([P, 4 * D], BF16, tag="hsb")
                    if pw_uniform and b1_zero:
                        nc.scalar.activation(hsb[:], psh[:], AF.Prelu,
                                             alpha=pwt[:, 0:1])
                    else:
                        for cb in range(4):
                            csl = slice(cb * D, (cb + 1) * D)
                            nc.scalar.activation(
                                hsb[:, csl], psh[:, csl], AF.Prelu,
                                bias=(0.0 if b1_zero else b1t[:, cb:cb + 1]),
                                alpha=pwt[:, cb:cb + 1])
                    psf = fps.tile([P, D], F32, tag="psf")
                    for cb in range(4):
                        nc.tensor.matmul(psf[:],
                                         lhsT=w2[:, cb * D:(cb + 1) * D],
                                         rhs=hsb[:, cb * D:(cb + 1) * D],
                                         start=(cb == 0), stop=(cb == 3))
                    fT = ftmp.tile([P, D], BF16, tag="fT")
                    if b2_zero:
                        nc.scalar.copy(fT[:], psf[:])
                    else:
                        nc.scalar.activation(fT[:], psf[:], AF.Identity,
                                             bias=b2t[:, 0:1])
                    psb = fps.tile([P, D], BF16, tag="psb")
                    nc.tensor.transpose(psb[:], fT[:], identb[:])
                    nc.vector.tensor_tensor(out=x2fblk[:, isl], in0=psb[:],
                                            in1=rstblk[:, isl], op=OP.add)
                    _ln_stats(nc, esm, x2fblk[:, isl], vepsB[:, i:i + 1],
                              meanB[:, i:i + 1])

                # batched LN2 + block output write
                rB = _ln_rsqrt(nc, eblk, vepsB[:, :nb], "B")
                bB = eblk.tile([P, NTB], F32, tag="bB")
                nc.vector.tensor_tensor(out=bB[:, :nb], in0=meanB[:, :nb],
                                        in1=rB[:], op=OP.mult)
                outblk = eblk.tile([P, NTB * D], F32, tag="outblk")
                for i in range(nb):
                    isl = slice(i * D, (i + 1) * D)
                    _ln_norm(nc, esm, x2fblk[:, isl], outblk[:, isl],
                             bB[:, i:i + 1], rB[:, i:i + 1],
                             ln_rep.get("ln2_g"), ln_rep.get("ln2_b"))
                nc.scalar.dma_start(
                    out_d[:, t0 * D:(t0 + nb) * D], outblk[:, :nb * D])

            NB = len(BLKS)
            emit_E(0)
            for blk in range(NB):
                if blk + 1 < NB:
                    emit_E(blk + 1)
                emit_rest(blk)

        resid.release()
        dramp.release()
        consts.release()

    nc.compile()
    return nc


def _ln_stats(nc, pool, x, veps_out, mean_out):
    """bn stats for one LN row-block; stashes var+eps and mean columns."""
    stats = pool.tile([P, nc.vector.BN_STATS_DIM], F32, tag="ln_stats")
    nc.vector.bn_stats(out=stats[:], in_=x)
    mv = pool.tile([P, nc.vector.BN_AGGR_DIM], F32, tag="ln_mv")
    nc.vector.bn_aggr(out=mv[:], in_=stats[:])
    nc.vector.tensor_scalar(out=veps_out, in0=mv[:, 1:2], scalar1=EPS,
                            scalar2=None, op0=OP.add)
    nc.scalar.activation(mean_out, mv[:, 0:1], AF.Copy, scale=-1.0)


def _ln_rsqrt(nc, pool, veps, tag):
    """r = (var+eps)^-0.5 via exp(-0.5*ln(.)), batched over a block."""
    lnv = pool.tile([P, veps.shape[1]], F32, tag=f"ln_lnv{tag}")
    nc.scalar.activation(lnv[:], veps[:], AF.Ln)
    r = pool.tile([P, veps.shape[1]], F32, tag=f"ln_r{tag}")
    nc.scalar.activation(r[:], lnv[:], AF.Exp, scale=-0.5)
    return r


def _ln_norm(nc, pool, x, out_ap, b, r, g_rep, b_rep):
    """normalize: out = x * r + (-mean * r) as one Activation (scale/bias)."""
    if g_rep is None and b_rep is None:
        nc.scalar.activation(out_ap, x, AF.Identity, bias=b, scale=r)
    else:
        y = pool.tile([P, D], F32, tag="ln_y")
        nc.scalar.activation(y[:], x, AF.Identity, bias=b, scale=r)
        if g_rep is not None:
            y2 = pool.tile([P, D], F32, tag="ln_y2")
            nc.vector.tensor_tensor(out=y2[:], in0=y[:], in1=g_rep[:],
                                    op=OP.mult)
            y = y2
        if b_rep is not None:
            nc.vector.tensor_tensor(out=out_ap, in0=y[:], in1=b_rep[:],
                                    op=OP.add)
        else:
            nc.vector.tensor_copy(out_ap, y[:])


_CACHE = {}


def _make_in_maps(feat, Wq, Wk, Wv, ln1_g, ln1_b, ln2_g, ln2_b,
                  W1, b1, prelu_w, W2, b2, caps, per_core):
    import ml_dtypes

    Wq = np.asarray(Wq, np.float32)[:, PERM]
    Wk = np.asarray(Wk, np.float32)[:, PERM]
    Wv = np.asarray(Wv, np.float32)[:, PERM]
    W1p = np.asarray(W1, np.float32)[PERM, :]
    W2p = np.asarray(W2, np.float32)[:, PERM]
    feat_p = feat[:, PERM]

    common = dict(
        wkv=np.ascontiguousarray(
            np.concatenate([Wk, Wv], axis=1)).astype(ml_dtypes.bfloat16),
        wq=np.ascontiguousarray(Wq).astype(ml_dtypes.bfloat16),
        w1=np.ascontiguousarray(W1p).astype(ml_dtypes.bfloat16),
        w2=np.ascontiguousarray(
            W2p.reshape(4, P, D).transpose(1, 0, 2).reshape(P, 4 * D)
        ).astype(ml_dtypes.bfloat16),
        b1t=np.ascontiguousarray(
            np.asarray(b1, np.float32).reshape(4, P).T),
        b2t=np.ascontiguousarray(
            np.asarray(b2, np.float32)[PERM].reshape(P, 1)),
        pwt=np.ascontiguousarray(
            np.asarray(prelu_w, np.float32).reshape(4, P).T),
        ln1_g=np.ascontiguousarray(np.asarray(ln1_g, np.float32)[PERM]),
        ln1_b=np.ascontiguousarray(np.asarray(ln1_b, np.float32)[PERM]),
        ln2_g=np.ascontiguousarray(np.asarray(ln2_g, np.float32)[PERM]),
        ln2_b=np.ascontiguousarray(np.asarray(ln2_b, np.float32)[PERM]),
    )

    cap_ao, cap_bt, cap_bo = caps
    tbl_rows = cap_ao + cap_bt + cap_bo
    featT_b = np.ascontiguousarray(feat.T).astype(ml_dtypes.bfloat16)
    in_maps = []
    for c in range(NCORES):
        pc = per_core[c]
        m = dict(common)
        m["eidx"] = pc["eidx"]
        m["npads"] = pc["npads"]
        tn = pc["tbl_nodes"]
        tblT = np.zeros((P, tbl_rows), ml_dtypes.bfloat16)
        valid = tn >= 0
        tblT[:, valid] = featT_b[:, tn[valid]]
        m["tblT"] = tblT
        m["fpm"] = np.ascontiguousarray(
            feat_p[pc["perm_in"]].reshape(P, NT * D))
        m["fpmT"] = np.ascontiguousarray(
            feat[pc["perm_in"].T.reshape(-1)].T).astype(ml_dtypes.bfloat16)
        in_maps.append(m)
    return in_maps


def kernel(feat, src, dst, Wq, Wk, Wv, ln1_g, ln1_b, ln2_g, ln2_b,
           W1, b1, prelu_w, W2, b2):
    from concourse.bass_utils import run_bass_kernel_spmd

    feat = np.ascontiguousarray(np.asarray(feat), dtype=np.float32)
    src = np.asarray(src).astype(np.int64)
    dst = np.asarray(dst).astype(np.int64)

    steps, caps, per_core = _host_prep(src, dst)
    ln_trivial1 = bool(np.all(ln1_g == 1.0) and np.all(ln1_b == 0.0))
    ln_trivial2 = bool(np.all(ln2_g == 1.0) and np.all(ln2_b == 0.0))
    b1_zero = bool(np.all(np.asarray(b1) == 0.0))
    b2_zero = bool(np.all(np.asarray(b2) == 0.0))
    pw_uniform = bool(np.all(np.asarray(prelu_w) == np.asarray(prelu_w)[0]))

    key = (tuple(steps), caps, ln_trivial1, ln_trivial2, b1_zero, b2_zero,
           pw_uniform)
    if key not in _CACHE:
        _CACHE[key] = _build_program(steps, caps, ln_trivial1, ln_trivial2,
                                     b1_zero, b2_zero, pw_uniform)
    nc = _CACHE[key]

    in_maps = _make_in_maps(feat, Wq, Wk, Wv, ln1_g, ln1_b, ln2_g, ln2_b,
                            W1, b1, prelu_w, W2, b2, caps, per_core)

    res = run_bass_kernel_spmd(nc, in_maps, list(range(NCORES)))
    global LAST_RESULT
    LAST_RESULT = res
    out = np.empty((N, D), np.float32)
    inv = np.empty(D, np.int64)
    inv[PERM] = np.arange(D)
    for c in range(NCORES):
        r = res.results[c]["out"].reshape(P, NT, D)
        po = per_core[c]["perm_out"]          # [P, NT] local ids (NLOC = dummy)
        valid = po < NLOC
        out[c * NLOC + po[valid]] = r[valid][:, inv]
    return out
